# revision 11
# baseline (speedup 1.0000x reference)
"""Trainium2 Bass kernel for nn_Model_24799141167781 (GNN message passing, 2x SpGAT).

8 NeuronCores, SPMD. Nodes degree-sorted + snake-dealt to cores (stripe of
S=6272 rows each). Per-node tables [h | f_dst] in fp16 DRAM rows, replicated
via AllGather. Edge messages fetched with dma_gather in a [128 nodes x K
slots] layout; int16 index range handled by an A/B table split at the core-5
boundary. e = exp(-lrelu(fs+fd)) via 2 ACT ops (accum_out gives the
denominator); pad slots hit a zero row with fd=3e4 so e underflows to 0.

v3: the axon-tunneled PJRT path charges ~1.7ms per input tensor per call,
~0.32ms/MB of input bytes, and ~0.9ms per collective; device compute is
almost free. Hence:
- ALL inputs packed into two per-core blobs (blob16/blob32), sliced on
  device via APs + bitcast
- gather index pieces shipped un-replicated ([16, w]) and expanded to 128
  partitions on device with 3 doubling DMAs
- collectives merged 9 -> 4: word + user-L1 tables share one AllGather
  (interleaved 256-col rows); the two L2 tables share one AllGather
  (512-col rows); the fusion AllReduce carries per-view partial logits
  [BT+P, 4] plus the attention numerator row, so the attention softmax and
  view mixing happen after a single reduce
- wide persistent SBUF row buffers; ONE rearranged DMA per stripe;
  batched EPS/reciprocal/ELU/log_softmax
"""

import os
import sys
from contextlib import ExitStack

import numpy as np

sys.path.insert(0, "/opt/trn_rl_repo")
os.environ["NEURON_SCRATCHPAD_PAGE_SIZE"] = "64"

import concourse.bass as bass
import concourse.mybir as mybir
import concourse.tile as tile
from concourse.masks import make_identity

F32 = mybir.dt.float32
F16 = mybir.dt.float16
I16 = mybir.dt.int16
I32 = mybir.dt.int32

NCORES = 8
P = 128
ALPHA = 0.2
EPS = 1e-16
PAD_FD = 30000.0
ACORES = 5


def _snake_deal(n):
    r = np.arange(n)
    c = r % (2 * NCORES)
    return np.where(c < NCORES, c, 2 * NCORES - 1 - c)


def _wrap16(flat_i16, pad_val):
    n = flat_i16.shape[0]
    s = max((n + 15) // 16, 1)
    buf = np.full(s * 16, pad_val, np.int16)
    buf[:n] = flat_i16
    return buf.reshape(s, 16).T  # [16, s]; device replicates to 128 rows


class SlotStruct:
    def __init__(self, rows_core, rows_local, cols_gid, S, ntiles, za, zb,
                 b_base):
        self.ntiles = ntiles
        half_b = cols_gid >= b_base
        key = (rows_core.astype(np.int64) * S * 2
               + rows_local.astype(np.int64) * 2 + half_b)
        order = np.argsort(key, kind="stable")
        k_s = key[order]
        col_s = cols_gid[order]
        halfb_s = half_b[order]
        core_s = rows_core[order]
        local_s = rows_local[order]
        grp_start = np.r_[0, np.flatnonzero(np.diff(k_s)) + 1]
        grp_len = np.diff(np.r_[grp_start, k_s.shape[0]])
        slot = np.arange(k_s.shape[0]) - np.repeat(grp_start, grp_len)

        tiles = local_s // P
        parts = local_s % P
        cntA = np.zeros((NCORES, ntiles), np.int64)
        cntB = np.zeros((NCORES, ntiles), np.int64)
        selA = ~halfb_s
        if selA.any():
            np.maximum.at(cntA, (core_s[selA], tiles[selA]), slot[selA] + 1)
        if (~selA).any():
            np.maximum.at(cntB, (core_s[~selA], tiles[~selA]), slot[~selA] + 1)
        self.KA = cntA.max(axis=0)
        self.KB = cntB.max(axis=0)
        self.offA = np.r_[0, np.cumsum(self.KA)]
        self.offB = np.r_[0, np.cumsum(self.KB)]
        totA, totB = int(self.offA[-1]), int(self.offB[-1])

        flatA = np.full((NCORES, max(totA, 1) * P), za, np.int32)
        flatB = np.full((NCORES, max(totB, 1) * P), zb - b_base, np.int32)
        posA = self.offA[tiles[selA]] * P + slot[selA] * P + parts[selA]
        flatA[core_s[selA], posA] = col_s[selA]
        posB = self.offB[tiles[~selA]] * P + slot[~selA] * P + parts[~selA]
        flatB[core_s[~selA], posB] = col_s[~selA] - b_base
        assert flatA.max() < 32768 and flatB.max() < 32768
        self.idxA = np.stack([_wrap16(flatA[c].astype(np.int16), za)
                              for c in range(NCORES)])
        self.idxB = np.stack(
            [_wrap16(flatB[c].astype(np.int16), np.int16(zb - b_base))
             for c in range(NCORES)])


def _dma_gather_flex(gp, out_ap, in_ap, idxs_ap, num_idxs, elem_size,
                     elem_step, single_packet=False):
    """InstDMAGatherAnt with elem_size_bytes not a multiple of 256 (the ucode
    only needs the row STRIDE 256B-aligned). in_ap must be col-sliced so its
    innermost dim count == elem_size and ap[0][0] == elem_step."""
    from concourse import ap_utils
    assert idxs_ap.dtype == mybir.dt.int16
    assert in_ap.dtype == out_ap.dtype
    assert ap_utils.ap_is_contiguous(out_ap.ap[1:])
    assert ap_utils.ap_is_contiguous(idxs_ap.ap[1:])
    assert in_ap.ap[-1][1] == elem_size and in_ap.ap[0][0] == elem_step
    stride_bytes = elem_step * mybir.dt.size(in_ap.dtype)
    assert stride_bytes % 256 == 0 and stride_bytes // 256 < 256
    _in_ap = gp.lower_ap_dma(in_ap, for_custom_bir_dma=True)
    _idxs_ap = gp.lower_ap(idxs_ap)
    _out_ap = gp.lower_ap(out_ap)
    return gp.add_instruction(
        mybir.InstDMAGatherAnt(
            name=gp.bass.get_next_instruction_name(),
            ins=[*_in_ap, _idxs_ap,
                 gp.lower_val_access(gp.to_reg(num_idxs))],
            outs=[_out_ap],
            transpose=False, num_idxs=num_idxs, elem_size=elem_size,
            stride_bytes_256=stride_bytes // 256, gen_mode=0,
            single_packet=single_packet, queue_num=0,
            sbuf_tokens_per_rank=0, sbuf_free_dim_per_rank=0,
            sbuf_free_dim_pad_per_rank=0, sbuf_byte_offset=0))


def host_prep(inputs):
    fi = np.asarray(inputs["features_index"])
    N = fi.shape[0]
    VOCAB = inputs["word_emb"].shape[0]
    NFEAT = inputs["word_emb"].shape[1]
    HID = inputs["tw_W1"].shape[1]
    JOINT = inputs["tw_W2"].shape[1]
    B = inputs["tw_graph_idx"].shape[0]
    assert N == VOCAB == inputs["user_emb"].shape[0]
    assert N % NCORES == 0
    npc = N // NCORES                      # real nodes per core
    S = ((npc + P - 1) // P) * P
    assert npc < S, "need pad rows per stripe"
    ntiles = S // P
    b_base = ACORES * S

    p = dict(N=N, S=S, ntiles=ntiles, B=B, NFEAT=NFEAT, HID=HID, JOINT=JOINT,
             b_base=b_base, npc=npc)

    def number_nodes(row, col, tertiary=None):
        deg = np.bincount(row, minlength=N)
        order = np.argsort(-deg, kind="stable")
        core_of = np.empty(N, np.int64)
        core_of[order] = _snake_deal(N)
        half_a = core_of[col] < ACORES
        degA = np.bincount(row[half_a], minlength=N)
        degB = deg - degA
        ter = tertiary if tertiary is not None else np.zeros(N, np.int64)
        local = np.empty(N, np.int64)
        for c in range(NCORES):
            mine = np.flatnonzero(core_of == c)
            o = mine[np.lexsort((ter[mine], degB[mine], degA[mine]))[::-1]]
            local[o] = np.arange(o.shape[0])
        return core_of, local, core_of * S + local

    tw_row = np.asarray(inputs["tw_edges"][0])
    tw_col = np.asarray(inputs["tw_edges"][1])
    ut_row = np.asarray(inputs["ut_edges"][0])
    ut_col = np.asarray(inputs["ut_edges"][1])
    # tertiary key for tweets: word-half-A count, to tighten the word-mean
    # A/B slot rectangles within (degA, degB) groups
    wA_cnt = (fi % NCORES < ACORES).sum(axis=1).astype(np.int64)
    twc, twl, twg = number_nodes(tw_row, tw_col, tertiary=wA_cnt)
    utc, utl, utg = number_nodes(ut_row, ut_col)
    p["twc"], p["twl"], p["utc"], p["utl"] = twc, twl, utc, utl

    za, zb = 0 * S + npc, ACORES * S + npc
    p["tw_slots"] = SlotStruct(twc[tw_row], twl[tw_row], twg[tw_col],
                               S, ntiles, za, zb, b_base)
    p["ut_slots"] = SlotStruct(utc[ut_row], utl[ut_row], utg[ut_col],
                               S, ntiles, za, zb, b_base)

    w = np.arange(VOCAB)
    wcore, wlocal = w % NCORES, w // NCORES
    gw = wcore * S + wlocal
    L = fi.shape[1]
    t_rep = np.repeat(np.arange(N), L)
    p["wm_slots"] = SlotStruct(twc[t_rep], twl[t_rep], gw[fi.reshape(-1)],
                               S, ntiles, za, zb, b_base)

    word_emb = np.asarray(inputs["word_emb"], np.float32)
    user_emb = np.asarray(inputs["user_emb"], np.float32)
    # host-transposed fp16 stripes [NFEAT, S] (row-permuted per core)
    wsT = np.zeros((NCORES, NFEAT, S), np.float16)
    usT = np.zeros((NCORES, NFEAT, S), np.float16)
    for c in range(NCORES):
        sel = np.flatnonzero(wcore == c)
        wsT[c][:, wlocal[sel]] = word_emb[sel].T.astype(np.float16)
        sel = np.flatnonzero(utc == c)
        usT[c][:, utl[sel]] = user_emb[sel].T.astype(np.float16)

    def fold1(W1, a1):
        h = W1.shape[1]
        return np.concatenate(
            [W1, W1 @ a1[h:, None], W1 @ a1[:h, None]], axis=1)

    p["tw_W1f"] = fold1(np.asarray(inputs["tw_W1"]),
                        np.asarray(inputs["tw_a1"])).astype(np.float16)
    p["tu_W1f"] = fold1(np.asarray(inputs["tu_W1"]),
                        np.asarray(inputs["tu_a1"])).astype(np.float16)
    p["tw_W2f"] = fold1(np.asarray(inputs["tw_W2"]),
                        np.asarray(inputs["tw_a2"])).astype(np.float16)
    p["tu_W2f"] = fold1(np.asarray(inputs["tu_W2"]),
                        np.asarray(inputs["tu_a2"])).astype(np.float16)
    p["weight_W"] = np.asarray(inputs["weight_W"]).astype(np.float16)
    p["projT"] = np.asarray(inputs["weight_proj"]).reshape(1, JOINT).astype(np.float32)
    p["out_WT"] = np.asarray(inputs["out_W"]).T.astype(np.float16)

    twi = np.asarray(inputs["tw_graph_idx"])
    uti = np.asarray(inputs["ut_graph_idx"])
    BT = ((B + P - 1) // P) * P + P
    p["BT"] = BT
    bt_tiles = BT // P
    p["bt_tiles"] = bt_tiles
    # bias tile replicated for the batched log_softmax pass
    p["obt"] = np.tile(np.asarray(inputs["out_b"], np.float32).reshape(1, 2),
                       (P, bt_tiles)).astype(np.float32)
    u_max = 1
    owns = []
    for c in range(NCORES):
        own = np.flatnonzero((twc[twi] == c) | (utc[uti] == c))
        owns.append(own)
        u_max = max(u_max, (own.shape[0] + P - 1) // P)
    p["u_fus"] = u_max
    g_tw = np.zeros((NCORES, 16, u_max * 8), np.int16)
    g_tu = np.zeros((NCORES, 16, u_max * 8), np.int16)
    sc_idx = np.zeros((NCORES, 128, u_max), np.int32)
    for c in range(NCORES):
        own = owns[c]
        n = own.shape[0]
        ftw = np.full(u_max * P, npc, np.int32)
        ftu = np.full(u_max * P, npc, np.int32)
        pos = np.arange(n)
        sel = twc[twi[own]] == c
        ftw[pos[sel]] = twl[twi[own[sel]]]
        sel = utc[uti[own]] == c
        ftu[pos[sel]] = utl[uti[own[sel]]]
        g_tw[c] = _wrap16(ftw.astype(np.int16), np.int16(npc))
        g_tu[c] = _wrap16(ftu.astype(np.int16), np.int16(npc))
        sc = B + np.tile(np.arange(P), u_max)
        sc[pos] = own
        sc_idx[c] = sc.reshape(u_max, P).T
    p["fus_gtw"], p["fus_gtu"], p["fus_sc"] = g_tw, g_tu, sc_idx

    # ---- pack all inputs into two per-core blobs (one 2-byte, one 4-byte):
    # the axon-tunneled PJRT path costs ~1.7ms per input tensor per call,
    # so input COUNT dominates the measured time. Pieces live as [rows, cols]
    # blocks at column offsets inside blob16 [128, W16] / blob32 [128, W32].
    kchunks = [(i, min(P, NFEAT - i)) for i in range(0, NFEAT, P)]
    L16, off = {}, 0

    def add16(key, rows, cols):
        nonlocal off
        L16[key] = (rows, cols, off)
        off += cols
    for ki, (k0, kn) in enumerate(kchunks):
        add16(f"wsT{ki}", kn, S)
    for ki, (k0, kn) in enumerate(kchunks):
        add16(f"usT{ki}", kn, S)
    for ki, (k0, kn) in enumerate(kchunks):
        add16(f"twW1f{ki}", kn, HID + 2)
    for ki, (k0, kn) in enumerate(kchunks):
        add16(f"tuW1f{ki}", kn, HID + 2)
    add16("twW2f", HID, JOINT + 2)
    add16("tuW2f", HID, JOINT + 2)
    add16("weightW", JOINT, JOINT)
    add16("outWT", JOINT, 2)
    add16("wmA", 16, p["wm_slots"].idxA.shape[2])
    add16("wmB", 16, p["wm_slots"].idxB.shape[2])
    add16("twA", 16, p["tw_slots"].idxA.shape[2])
    add16("twB", 16, p["tw_slots"].idxB.shape[2])
    add16("utA", 16, p["ut_slots"].idxA.shape[2])
    add16("utB", 16, p["ut_slots"].idxB.shape[2])
    add16("fgtw", 16, u_max * 8)
    add16("fgtu", 16, u_max * 8)
    W16 = off
    L32, off = {}, 0

    def add32(key, rows, cols):
        nonlocal off
        L32[key] = (rows, cols, off)
        off += cols
    add32("projT", 1, JOINT)
    add32("obt", P, bt_tiles * 2)
    add32("fussc", 128, u_max)
    W32 = off
    p["L16"], p["W16"], p["L32"], p["W32"] = L16, W16, L32, W32

    blob16 = np.zeros((NCORES, 128, W16), np.int16)
    blob32 = np.zeros((NCORES, 128, W32), np.int32)

    def put16(c, key, arr):
        r, w, o = L16[key]
        assert arr.shape == (r, w), (key, arr.shape, (r, w))
        blob16[c, :r, o:o + w] = arr.view(np.int16)

    def put32(c, key, arr):
        r, w, o = L32[key]
        assert arr.shape == (r, w), (key, arr.shape, (r, w))
        blob32[c, :r, o:o + w] = arr.view(np.int32)

    for c in range(NCORES):
        for ki, (k0, kn) in enumerate(kchunks):
            put16(c, f"wsT{ki}", wsT[c][k0:k0 + kn])
            put16(c, f"usT{ki}", usT[c][k0:k0 + kn])
            put16(c, f"twW1f{ki}", p["tw_W1f"][k0:k0 + kn])
            put16(c, f"tuW1f{ki}", p["tu_W1f"][k0:k0 + kn])
        put16(c, "twW2f", p["tw_W2f"])
        put16(c, "tuW2f", p["tu_W2f"])
        put16(c, "weightW", p["weight_W"])
        put16(c, "outWT", p["out_WT"])
        put16(c, "wmA", p["wm_slots"].idxA[c])
        put16(c, "wmB", p["wm_slots"].idxB[c])
        put16(c, "twA", p["tw_slots"].idxA[c])
        put16(c, "twB", p["tw_slots"].idxB[c])
        put16(c, "utA", p["ut_slots"].idxA[c])
        put16(c, "utB", p["ut_slots"].idxB[c])
        put16(c, "fgtw", p["fus_gtw"][c])
        put16(c, "fgtu", p["fus_gtu"][c])
        put32(c, "projT", p["projT"])
        put32(c, "obt", p["obt"])
        put32(c, "fussc", p["fus_sc"][c])
    p["blob16"], p["blob32"] = blob16, blob32
    return p


def build_program(p):
    import concourse.bacc as bacc
    nc_b = bacc.Bacc("TRN2", target_bir_lowering=False, debug=False,
                     num_devices=NCORES)
    tcx = tile.TileContext(nc_b)
    S, ntiles, B, BT = p["S"], p["ntiles"], p["B"], p["BT"]
    NFEAT, HID, JOINT, N = p["NFEAT"], p["HID"], p["JOINT"], p["N"]
    b_base, npc = p["b_base"], p["npc"]
    NT = NCORES * S
    DW, DL2 = HID * 2, JOINT * 2
    DC1, DC2 = 2 * DW, 2 * DL2           # merged-table row widths
    u_fus = p["u_fus"]
    bt_tiles = p["bt_tiles"]
    BT2 = BT + P
    wm, tws, uts = p["wm_slots"], p["tw_slots"], p["ut_slots"]
    npad = S - npc
    kchunks = [(i, min(P, NFEAT - i)) for i in range(0, NFEAT, P)]
    nk = len(kchunks)
    # embeddings streamed in two tile-aligned half-stripes to cap SBUF use
    th_splits = [(0, (ntiles + 1) // 2), ((ntiles + 1) // 2, ntiles)]
    Shmax = max(t1 - t0 for t0, t1 in th_splits) * P

    with tcx as tc:
        nc = tc.nc
        ctx = ExitStack()

        def inp(name, shape, dtype):
            return nc.dram_tensor(name, shape, dtype, kind="ExternalInput").ap()

        def internal(name, shape, dtype, shared=False):
            return nc.dram_tensor(
                name, shape, dtype, kind="Internal",
                addr_space="Shared" if shared else "Local").ap()

        blob16 = inp("blob16", [128, p["W16"]], I16)
        blob32 = inp("blob32", [128, p["W32"]], I32)
        L16, L32 = p["L16"], p["L32"]

        def b16(key, dt=F16):
            r, w, o = L16[key]
            ap = blob16[0:r, o:o + w]
            return ap if dt == I16 else ap.bitcast(dt)

        def b32(key, dt=F32):
            r, w, o = L32[key]
            ap = blob32[0:r, o:o + w]
            return ap if dt == I32 else ap.bitcast(dt)

        out = nc.dram_tensor("out", [B, 2], F32, kind="ExternalOutput").ap()

        # merged tables: comb1 rows = [word row (128) | user-L1 row (128)],
        # comb2 rows = [ut-L2 row (256) | tw-L2 row (256)]
        comb1_stripe = internal("comb1_s", [S, DC1], F16)
        comb1_table = internal("comb1_t", [NT, DC1], F16, shared=True)
        t1tw_stripe = internal("t1tw_s", [S, DW], F16)
        t1tw_table = internal("t1tw_t", [NT, DW], F16, shared=True)
        comb2_stripe = internal("comb2_s", [S, DC2], F16)
        comb2_table = internal("comb2_t", [NT, DC2], F16, shared=True)
        x_stripe = {g: internal(f"{g}_x", [S, JOINT], F16) for g in ("tw", "ut")}
        lbuf = internal("lbuf", [BT2, 4], F32)
        lbuf_r = internal("lbuf_r", [BT2, 4], F32, shared=True)

        rg = [list(range(NCORES))]

        big = ctx.enter_context(tc.tile_pool(name="big", bufs=2))
        med = ctx.enter_context(tc.tile_pool(name="med", bufs=4))
        sml = ctx.enter_context(tc.tile_pool(name="sml", bufs=6))
        wide = ctx.enter_context(tc.tile_pool(name="wide", bufs=1))
        fsp = ctx.enter_context(tc.tile_pool(name="fsp", bufs=8))
        pst = ctx.enter_context(tc.tile_pool(name="pst", bufs=2, space="PSUM"))
        psm = ctx.enter_context(tc.tile_pool(name="psm", bufs=2, space="PSUM"))
        acc = ctx.enter_context(tc.tile_pool(name="acc", bufs=1, space="PSUM"))
        cst = ctx.enter_context(tc.tile_pool(name="cst", bufs=1))
        idxall = ctx.enter_context(tc.tile_pool(name="idxall", bufs=1))

        ident16 = cst.tile([P, P], F16, tag="ident16")
        make_identity(nc, ident16[:])
        ones_col = cst.tile([P, 1], F16, tag="ones_col")
        nc.vector.memset(ones_col[:], 1.0)
        ones_row = cst.tile([1, P], F16, tag="ones_row")
        nc.vector.memset(ones_row[:], 1.0)
        padfd = cst.tile([P, 1], F16, tag="padfd")
        nc.vector.memset(padfd[:], PAD_FD)

        def expand128(dst, src_ap, w):
            # replicate [16, w] int16 into [128, w] via doubling DMAs
            nc.sync.dma_start(dst[0:16, 0:w], src_ap)
            nc.sync.dma_start(dst[16:32, 0:w], dst[0:16, 0:w])
            nc.sync.dma_start(dst[32:64, 0:w], dst[0:32, 0:w])
            nc.sync.dma_start(dst[64:128, 0:w], dst[0:64, 0:w])

        def load_idx(slots, keyA, keyB, tagsfx=""):
            wa = int(slots.offA[-1]) * 8
            wb = int(slots.offB[-1]) * 8
            ia = idxall.tile([P, max(wa, 8)], I16, tag="ia_all",
                             name="ia_all" + tagsfx)
            if wa > 0:
                expand128(ia, b16(keyA, I16)[:, 0:wa], wa)
            ib = idxall.tile([P, max(wb, 8)], I16, tag="ib_all",
                             name="ib_all" + tagsfx)
            if wb > 0:
                expand128(ib, b16(keyB, I16)[:, 0:wb], wb)
            return ia, ib

        def slot_gather(slots, ia, ib, table, col0, dtab, dg, t):
            # dg = payload elems per row; rows start at column col0 of the
            # dtab-elem-wide table rows
            KA, KB = int(slots.KA[t]), int(slots.KB[t])
            K = max(KA + KB, 1)
            g_t = big.tile([P, K, dg], F16, tag="g_e", bufs=2)
            if KA + KB == 0:
                nc.vector.memset(g_t[:], 0.0)
                return g_t, K
            if KA > 0:
                _dma_gather_flex(
                    nc.gpsimd, g_t[:, 0:KA, :],
                    table[0:b_base, col0:col0 + dg],
                    ia[:, int(slots.offA[t]) * 8:int(slots.offA[t] + KA) * 8],
                    KA * P, dg, dtab, single_packet=(KA * P <= 1024))
            if KB > 0:
                _dma_gather_flex(
                    nc.gpsimd, g_t[:, KA:KA + KB, :],
                    table[b_base:, col0:col0 + dg],
                    ib[:, int(slots.offB[t]) * 8:int(slots.offB[t] + KB) * 8],
                    KB * P, dg, dtab, single_packet=(KB * P <= 1024))
            return g_t, K

        def elu_batched(buf_ap, nelem):
            # in-place ELU on an f16 [P, nelem] view
            m16 = med.tile([P, nelem], F16, tag="elu_m", bufs=1)
            nc.vector.tensor_scalar_min(m16[:], buf_ap, 0.0)
            nc.scalar.activation(m16[:], m16[:],
                                 mybir.ActivationFunctionType.Exp)
            nc.vector.tensor_scalar_add(m16[:], m16[:], -1.0)
            nc.vector.tensor_tensor(buf_ap, buf_ap, m16[:],
                                    op=mybir.AluOpType.max)

        def stripe_write(stripe, rows_ap, d):
            # one DMA: SBUF rows [P, ntiles, d] -> DRAM stripe [S, d]
            nc.sync.dma_start(
                stripe.rearrange("(t q) d -> q t d", q=P),
                rows_ap.rearrange("q (t d) -> q t d", d=d))

        # ===== phase 1: word + user-L1 rows -> merged comb1 table =====
        c1rows = wide.tile([P, ntiles * DC1], F16, tag="cX", name="c1rows")
        nc.vector.memset(c1rows[:], 0.0)
        fs1 = {}
        embp_ctx = ExitStack()
        embp = embp_ctx.enter_context(tc.tile_pool(name="embp", bufs=1))
        for g, ekey, wkey, col0, store_fs in (
                ("w", "wsT", "twW1f", 0, False),
                ("ut", "usT", "tuW1f", DW, True)):
            ncols = HID + 2
            wt = cst.tile([P, ncols * nk], F16, tag=f"wf_{wkey}")
            for ki, (k0, kn) in enumerate(kchunks):
                nc.sync.dma_start(wt[:kn, ki * ncols:(ki + 1) * ncols],
                                  b16(f"{wkey}{ki}"))
            if store_fs:
                fs1["ut"] = fsp.tile([P, ntiles], F32, tag="fs",
                                     name="fs1_ut")
            ncopy = ncols if not store_fs else ncols - 1
            for h, (t0, t1) in enumerate(th_splits):
                Swin = (t1 - t0) * P
                embT_sb = embp.tile([P, nk * Shmax], F16, tag="embT",
                                    name=f"embT_{g}{h}")
                for ki, (k0, kn) in enumerate(kchunks):
                    nc.sync.dma_start(
                        embT_sb[:kn, ki * Shmax:ki * Shmax + Swin],
                        b16(f"{ekey}{ki}")[:, t0 * P:t1 * P])
                for t in range(t0, t1):
                    tl = (t - t0) * P
                    ps = psm.tile([P, ncols], F32, tag="mm")
                    for ki, (k0, kn) in enumerate(kchunks):
                        nc.tensor.matmul(
                            ps[:],
                            embT_sb[:kn, ki * Shmax + tl:ki * Shmax + tl + P],
                            wt[:kn, ki * ncols:(ki + 1) * ncols],
                            start=(ki == 0), stop=(ki == nk - 1))
                    nc.vector.tensor_copy(
                        c1rows[:, t * DC1 + col0:t * DC1 + col0 + ncopy],
                        ps[:, 0:ncopy])
                    if store_fs:
                        nc.vector.tensor_copy(fs1["ut"][:, t:t + 1],
                                              ps[:, ncols - 1:ncols])
        embp_ctx.close()
        stripe_write(comb1_stripe, c1rows[:], DC1)
        nc.sync.dma_start(comb1_stripe[npc:S, DW + HID:DW + HID + 1],
                          padfd[:npad, :])
        nc.gpsimd.collective_compute("AllGather", mybir.AluOpType.bypass, rg,
                                     ins=[comb1_stripe[:]],
                                     outs=[comb1_table[:]])

        # ===== phase 2: tweet means -> tweet L1 stripe (own AllGather) =====
        wm_ia, wm_ib = load_idx(wm, "wmA", "wmB", "w")
        fs1["tw"] = fsp.tile([P, ntiles], F32, tag="fs", name="fs1_tw")
        rows_t = wide.tile([P, ntiles * DW], F16, tag="w12", name="rows_tw1")
        nc.vector.memset(rows_t[:], 0.0)
        for t in range(ntiles):
            g_t, K = slot_gather(wm, wm_ia, wm_ib, comb1_table, 0, DC1,
                                 HID + 2, t)
            mean = med.tile([P, HID + 2], F32, tag="wm_mean")
            nc.vector.tensor_reduce(
                mean[:], g_t[:].rearrange("q k d -> q d k"),
                axis=mybir.AxisListType.X, op=mybir.AluOpType.add)
            nc.vector.tensor_scalar_mul(
                rows_t[:, t * DW:t * DW + HID + 1], mean[:, :HID + 1],
                1.0 / 16.0)
            nc.vector.tensor_scalar_mul(fs1["tw"][:, t:t + 1],
                                        mean[:, HID + 1:HID + 2], 1.0 / 16.0)
        stripe_write(t1tw_stripe, rows_t[:], DW)
        nc.sync.dma_start(t1tw_stripe[npc:S, HID:HID + 1], padfd[:npad, :])
        nc.gpsimd.collective_compute("AllGather", mybir.AluOpType.bypass, rg,
                                     ins=[t1tw_stripe[:]],
                                     outs=[t1tw_table[:]])

        # ===== phase 3: both graphs' L1 pass -> merged comb2 table =====
        w2tiles = {}
        for g, w2key in (("ut", "tuW2f"), ("tw", "twW2f")):
            wt2 = cst.tile([P, JOINT + 2], F16, tag=f"w2_{g}")
            nc.sync.dma_start(wt2[:HID, :], b16(w2key))
            w2tiles[g] = wt2
        wwt = cst.tile([P, JOINT], F16, tag="wwt")
        nc.sync.dma_start(wwt[:], b16("weightW"))
        projs = cst.tile([1, JOINT], F32, tag="projs")
        nc.sync.dma_start(projs[:], b32("projT"))
        colsum = {g: acc.tile([1, JOINT], F32, tag=f"cs_{g}", name=f"cs_{g}")
                  for g in ("ut", "tw")}

        def attn_pass(slots, ia, ib, table, col0, dtab, d_in, fs_t, den_t,
                      num_t):
            for t in range(ntiles):
                g_t, K = slot_gather(slots, ia, ib, table, col0, dtab,
                                     d_in + 2, t)
                lr = med.tile([P, K], F32, tag="lr_e")
                fd_view = g_t[:, :, d_in:d_in + 1].rearrange(
                    "q k o -> q (k o)")
                nc.scalar.activation(lr[:], fd_view,
                                     mybir.ActivationFunctionType.Lrelu,
                                     bias=fs_t[:, t:t + 1], scale=1.0,
                                     alpha=ALPHA)
                e_t = med.tile([P, K], F16, tag="e_e")
                nc.scalar.activation(e_t[:], lr[:],
                                     mybir.ActivationFunctionType.Exp,
                                     scale=-1.0,
                                     accum_out=den_t[:, t:t + 1])
                nc.vector.tensor_tensor(g_t[:, :, 0:d_in],
                                        g_t[:, :, 0:d_in],
                                        e_t[:].to_broadcast([P, K, d_in]),
                                        op=mybir.AluOpType.mult)
                nc.vector.tensor_reduce(
                    num_t[:, t * d_in:(t + 1) * d_in],
                    g_t[:, :, 0:d_in].rearrange("q k d -> q d k"),
                    axis=mybir.AxisListType.X, op=mybir.AluOpType.add)

        def finish_o16(den_t, num_t, d_in, o16_t):
            nc.vector.tensor_scalar_add(den_t[:], den_t[:], EPS)
            nc.vector.reciprocal(den_t[:], den_t[:])
            for t in range(ntiles):
                nc.vector.tensor_scalar_mul(
                    o16_t[:, t * d_in:(t + 1) * d_in],
                    num_t[:, t * d_in:(t + 1) * d_in], den_t[:, t:t + 1])
            elu_batched(o16_t[:], ntiles * d_in)

        c2rows = wide.tile([P, ntiles * DC2], F16, tag="cX", name="c2rows")
        nc.vector.memset(c2rows[:], 0.0)
        eidx = {}
        fs2 = {}
        for g, keyA, keyB, slots, t1c0, t1tab, t1d in (
                ("ut", "utA", "utB", uts, DW, comb1_table, DC1),
                ("tw", "twA", "twB", tws, 0, t1tw_table, DW)):
            c2off = 0 if g == "ut" else DL2
            e_ia, e_ib = load_idx(slots, keyA, keyB, "e" + g)
            eidx[g] = (keyA, keyB)
            den1 = fsp.tile([P, ntiles], F32, tag="fs", name=f"den1_{g}")
            num1 = wide.tile([P, ntiles * HID], F32, tag="w12",
                             name=f"num1_{g}")
            attn_pass(slots, e_ia, e_ib, t1tab, t1c0, t1d, HID,
                      fs1[g], den1, num1)
            o16 = wide.tile([P, ntiles * HID], F16, tag="oX",
                            name=f"o16_{g}")
            finish_o16(den1, num1, HID, o16)
            fs2[g] = fsp.tile([P, ntiles], F32, tag="fs", name=f"fs2_{g}")
            for t in range(ntiles):
                tp = pst.tile([P, P], F16, tag="tp")
                nc.tensor.transpose(tp[:HID, :],
                                    o16[:, t * HID:(t + 1) * HID],
                                    ident16[:])
                tp16 = med.tile([P, P], F16, tag="tp16")
                nc.vector.tensor_copy(tp16[:HID, :], tp[:HID, :])
                ps2 = psm.tile([P, JOINT + 2], F32, tag="mm")
                nc.tensor.matmul(ps2[:], tp16[:HID, :], w2tiles[g][:HID, :],
                                 start=True, stop=True)
                nc.vector.tensor_copy(
                    c2rows[:, t * DC2 + c2off:t * DC2 + c2off + JOINT + 1],
                    ps2[:, :JOINT + 1])
                nc.vector.tensor_copy(fs2[g][:, t:t + 1],
                                      ps2[:, JOINT + 1:JOINT + 2])
        stripe_write(comb2_stripe, c2rows[:], DC2)
        nc.sync.dma_start(comb2_stripe[npc:S, JOINT:JOINT + 1],
                          padfd[:npad, :])
        nc.sync.dma_start(comb2_stripe[npc:S, DL2 + JOINT:DL2 + JOINT + 1],
                          padfd[:npad, :])
        nc.gpsimd.collective_compute("AllGather", mybir.AluOpType.bypass, rg,
                                     ins=[comb2_stripe[:]],
                                     outs=[comb2_table[:]])

        # ===== phase 4: both graphs' L2 pass =====
        for g, slots in (("ut", uts), ("tw", tws)):
            c2off = 0 if g == "ut" else DL2
            keyA, keyB = eidx[g]
            e_ia, e_ib = load_idx(slots, keyA, keyB, "e2" + g)
            den2 = fsp.tile([P, ntiles], F32, tag="fs", name=f"den2_{g}")
            num2 = wide.tile([P, ntiles * JOINT], F32, tag="w25",
                             name=f"num2_{g}")
            attn_pass(slots, e_ia, e_ib, comb2_table, c2off, DC2, JOINT,
                      fs2[g], den2, num2)
            x16 = wide.tile([P, ntiles * JOINT], F16, tag="oX",
                            name=f"x16_{g}")
            finish_o16(den2, num2, JOINT, x16)
            stripe_write(x_stripe[g], x16[:], JOINT)
            for t in range(ntiles):
                tp = pst.tile([P, P], F16, tag="tp")
                nc.tensor.transpose(tp[:], x16[:, t * JOINT:(t + 1) * JOINT],
                                    ident16[:])
                tp16 = med.tile([P, P], F16, tag="tp16")
                nc.vector.tensor_copy(tp16[:], tp[:])
                ups = psm.tile([P, JOINT], F32, tag="mm")
                nc.tensor.matmul(ups[:], tp16[:], wwt[:], start=True,
                                 stop=True)
                th = med.tile([P, JOINT], F16, tag="tanh")
                nc.scalar.activation(th[:], ups[:],
                                     mybir.ActivationFunctionType.Tanh)
                nc.tensor.matmul(colsum[g][:], ones_col[:], th[:],
                                 start=(t == 0), stop=(t == ntiles - 1),
                                 skip_group_check=True)

        # ===== phase 5: attention numerators (local partials) =====
        attp = sml.tile([1, 2], F32, tag="attp")
        for gi, g in enumerate(("tw", "ut")):
            prod = sml.tile([1, JOINT], F32, tag="pr", bufs=1)
            nc.vector.tensor_tensor(prod[:], colsum[g][:], projs[:],
                                    op=mybir.AluOpType.mult)
            nc.vector.tensor_reduce(attp[:, gi:gi + 1], prod[:],
                                    axis=mybir.AxisListType.X,
                                    op=mybir.AluOpType.add)
        nc.vector.tensor_scalar_mul(attp[:], attp[:], 1.0 / N)

        # ===== phase 6: per-view partial logits + att row -> ONE AllReduce
        zt = med.tile([P, (bt_tiles + 1) * 4], F32, tag="zt", bufs=1)
        nc.vector.memset(zt[:], 0.0)
        nc.sync.dma_start(lbuf.rearrange("(t q) d -> q t d", q=P),
                          zt[:].rearrange("q (t d) -> q t d", d=4))
        fgw = sml.tile([P, u_fus * 8], I16, tag="fgw")
        expand128(fgw, b16("fgtw", I16), u_fus * 8)
        fgu = sml.tile([P, u_fus * 8], I16, tag="fgu")
        expand128(fgu, b16("fgtu", I16), u_fus * 8)
        g1 = big.tile([P, u_fus, JOINT], F16, tag="fg1", bufs=1)
        nc.gpsimd.dma_gather(g1[:], x_stripe["tw"][:], fgw[:], u_fus * P,
                             u_fus * P, JOINT,
                             single_packet=(u_fus * P <= 1024))
        g2 = big.tile([P, u_fus, JOINT], F16, tag="fg2", bufs=1)
        nc.gpsimd.dma_gather(g2[:], x_stripe["ut"][:], fgu[:], u_fus * P,
                             u_fus * P, JOINT,
                             single_packet=(u_fus * P <= 1024))
        owt = cst.tile([P, 2], F16, tag="owt")
        nc.sync.dma_start(owt[:JOINT, :], b16("outWT"))
        sct = sml.tile([P, u_fus], I32, tag="fsct")
        nc.sync.dma_start(sct[:], b32("fussc", I32))
        for j in range(u_fus):
            lgs4 = sml.tile([P, 4], F32, tag="lgs4")
            for col, gsrc in ((0, g1), (2, g2)):
                tp = pst.tile([P, P], F16, tag="tp")
                nc.tensor.transpose(tp[:], gsrc[:, j, :], ident16[:])
                tp16 = med.tile([P, P], F16, tag="tp16")
                nc.vector.tensor_copy(tp16[:], tp[:])
                lg = psm.tile([P, 2], F32, tag="mm2")
                nc.tensor.matmul(lg[:], tp16[:JOINT, :], owt[:JOINT, :],
                                 start=True, stop=True)
                nc.vector.tensor_copy(lgs4[:, col:col + 2], lg[:])
            nc.gpsimd.indirect_dma_start(
                out=lbuf[:],
                out_offset=bass.IndirectOffsetOnAxis(ap=sct[:, j:j + 1],
                                                     axis=0),
                in_=lgs4[:], in_offset=None)
        nc.sync.dma_start(lbuf[BT:BT + 1, 0:2], attp[:])
        nc.gpsimd.collective_compute("AllReduce", mybir.AluOpType.add, rg,
                                     ins=[lbuf[:]], outs=[lbuf_r[:]])

        # ===== phase 7: att softmax + mix + bias + log_softmax, 1 out DMA
        atts = sml.tile([1, 2], F32, tag="atts")
        nc.sync.dma_start(atts[:], lbuf_r[BT:BT + 1, 0:2])
        mx = sml.tile([1, 1], F32, tag="attmx")
        nc.vector.tensor_reduce(mx[:], atts[:], axis=mybir.AxisListType.X,
                                op=mybir.AluOpType.max)
        sh = sml.tile([1, 2], F32, tag="attsh")
        nc.vector.tensor_scalar(sh[:], atts[:], mx[:], None,
                                op0=mybir.AluOpType.subtract)
        ex = sml.tile([1, 2], F32, tag="attex")
        nc.scalar.activation(ex[:], sh[:], mybir.ActivationFunctionType.Exp)
        sm = sml.tile([1, 1], F32, tag="attsm")
        nc.vector.tensor_reduce(sm[:], ex[:], axis=mybir.AxisListType.X,
                                op=mybir.AluOpType.add)
        nc.vector.reciprocal(sm[:], sm[:])
        att2 = sml.tile([1, 2], F16, tag="att2")
        nc.vector.tensor_scalar_mul(att2[:], ex[:], sm[:])
        attb_ps = psm.tile([P, 2], F32, tag="mm2")
        nc.tensor.matmul(attb_ps[:], ones_row[:], att2[:], start=True,
                         stop=True)
        attb = sml.tile([P, 2], F32, tag="attb")
        nc.vector.tensor_copy(attb[:], attb_ps[:])

        lgall = med.tile([P, bt_tiles, 4], F32, tag="lgall", bufs=1)
        nc.sync.dma_start(
            lgall[:],
            lbuf_r.rearrange("(t q) d -> q t d", q=P)[:, 0:bt_tiles, :])
        fin = med.tile([P, bt_tiles, 2], F32, tag="fin")
        nc.vector.tensor_scalar_mul(fin[:], lgall[:, :, 0:2], attb[:, 0:1])
        fin2 = med.tile([P, bt_tiles, 2], F32, tag="fin2")
        nc.vector.tensor_scalar_mul(fin2[:], lgall[:, :, 2:4], attb[:, 1:2])
        nc.vector.tensor_tensor(fin[:], fin[:], fin2[:],
                                op=mybir.AluOpType.add)
        obt = sml.tile([P, bt_tiles * 2], F32, tag="obt", bufs=1)
        nc.sync.dma_start(obt[:], b32("obt"))
        nc.vector.tensor_tensor(
            fin[:], fin[:], obt[:].rearrange("q (t d) -> q t d", d=2),
            op=mybir.AluOpType.add)
        m = med.tile([P, bt_tiles], F32, tag="lgm")
        nc.vector.tensor_reduce(m[:], fin[:], axis=mybir.AxisListType.X,
                                op=mybir.AluOpType.max)
        shl = med.tile([P, bt_tiles, 2], F32, tag="lgsh")
        nc.vector.tensor_tensor(shl[:], fin[:],
                                m[:].to_broadcast([P, bt_tiles, 2]),
                                op=mybir.AluOpType.subtract)
        exl = med.tile([P, bt_tiles, 2], F32, tag="lgex")
        nc.scalar.activation(exl[:], shl[:],
                             mybir.ActivationFunctionType.Exp)
        se = med.tile([P, bt_tiles], F32, tag="lgse")
        nc.vector.tensor_reduce(se[:], exl[:], axis=mybir.AxisListType.X,
                                op=mybir.AluOpType.add)
        ln = med.tile([P, bt_tiles], F32, tag="lgln")
        nc.scalar.activation(ln[:], se[:], mybir.ActivationFunctionType.Ln)
        res = med.tile([P, bt_tiles, 2], F32, tag="lgres")
        nc.vector.tensor_tensor(res[:], shl[:],
                                ln[:].to_broadcast([P, bt_tiles, 2]),
                                op=mybir.AluOpType.subtract)
        nc.sync.dma_start(out.rearrange("(t q) d -> q t d", q=P),
                          res[:, 0:B // P, :])

        ctx.close()
    return tcx


def _in_maps(p):
    return [{"blob16": p["blob16"][c], "blob32": p["blob32"][c]}
            for c in range(NCORES)]


def kernel(**inputs):
    from concourse import bass_utils
    p = host_prep(inputs)
    tcx = build_program(p)
    tcx.nc.compile()
    res = bass_utils.run_bass_kernel_spmd(tcx.nc, _in_maps(p),
                                          core_ids=list(range(NCORES)))
    return np.asarray(res.results[0]["out"], np.float32)


# revision 12
# speedup vs baseline: 1.2609x; 1.2609x over previous
"""Trainium2 Bass kernel for nn_Model_24799141167781 (GNN message passing, 2x SpGAT).

8 NeuronCores, SPMD. Nodes degree-sorted + snake-dealt to cores (stripe of
S=6272 rows each). Per-node tables [h | f_dst] in fp16 DRAM rows, replicated
via AllGather. Edge messages fetched with dma_gather in a [128 nodes x K
slots] layout; int16 index range handled by an A/B table split at the core-5
boundary. e = exp(-lrelu(fs+fd)) via 2 ACT ops (accum_out gives the
denominator); pad slots hit a zero row with fd=3e4 so e underflows to 0.

v3: the axon-tunneled PJRT path charges ~1.7ms per input tensor per call,
~0.32ms/MB of input bytes, and ~0.9ms per collective; device compute is
almost free. Hence:
- ALL inputs packed into two per-core blobs (blob16/blob32), sliced on
  device via APs + bitcast
- gather index pieces shipped un-replicated ([16, w]) and expanded to 128
  partitions on device with 3 doubling DMAs
- collectives merged 9 -> 4: word + user-L1 tables share one AllGather
  (interleaved 256-col rows); the two L2 tables share one AllGather
  (512-col rows); the fusion AllReduce carries per-view partial logits
  [BT+P, 4] plus the attention numerator row, so the attention softmax and
  view mixing happen after a single reduce
- wide persistent SBUF row buffers; ONE rearranged DMA per stripe;
  batched EPS/reciprocal/ELU/log_softmax
"""

import os
import sys
from contextlib import ExitStack

import numpy as np

sys.path.insert(0, "/opt/trn_rl_repo")
os.environ["NEURON_SCRATCHPAD_PAGE_SIZE"] = "64"

import concourse.bass as bass
import concourse.mybir as mybir
import concourse.tile as tile
from concourse.masks import make_identity

F32 = mybir.dt.float32
F16 = mybir.dt.float16
I16 = mybir.dt.int16
I32 = mybir.dt.int32

NCORES = 8
P = 128
ALPHA = 0.2
EPS = 1e-16
PAD_FD = 30000.0
ACORES = 5


def _snake_deal(n):
    r = np.arange(n)
    c = r % (2 * NCORES)
    return np.where(c < NCORES, c, 2 * NCORES - 1 - c)


def _wrap16(flat_i16, pad_val):
    n = flat_i16.shape[0]
    s = max((n + 15) // 16, 1)
    buf = np.full(s * 16, pad_val, np.int16)
    buf[:n] = flat_i16
    return buf.reshape(s, 16).T  # [16, s]; device replicates to 128 rows


class SlotStruct:
    def __init__(self, rows_core, rows_local, cols_gid, S, ntiles, za, zb,
                 b_base):
        self.ntiles = ntiles
        half_b = cols_gid >= b_base
        key = (rows_core.astype(np.int64) * S * 2
               + rows_local.astype(np.int64) * 2 + half_b)
        order = np.argsort(key, kind="stable")
        k_s = key[order]
        col_s = cols_gid[order]
        halfb_s = half_b[order]
        core_s = rows_core[order]
        local_s = rows_local[order]
        grp_start = np.r_[0, np.flatnonzero(np.diff(k_s)) + 1]
        grp_len = np.diff(np.r_[grp_start, k_s.shape[0]])
        slot = np.arange(k_s.shape[0]) - np.repeat(grp_start, grp_len)

        tiles = local_s // P
        parts = local_s % P
        cntA = np.zeros((NCORES, ntiles), np.int64)
        cntB = np.zeros((NCORES, ntiles), np.int64)
        selA = ~halfb_s
        if selA.any():
            np.maximum.at(cntA, (core_s[selA], tiles[selA]), slot[selA] + 1)
        if (~selA).any():
            np.maximum.at(cntB, (core_s[~selA], tiles[~selA]), slot[~selA] + 1)
        self.KA = cntA.max(axis=0)
        self.KB = cntB.max(axis=0)
        self.offA = np.r_[0, np.cumsum(self.KA)]
        self.offB = np.r_[0, np.cumsum(self.KB)]
        totA, totB = int(self.offA[-1]), int(self.offB[-1])

        flatA = np.full((NCORES, max(totA, 1) * P), za, np.int32)
        flatB = np.full((NCORES, max(totB, 1) * P), zb - b_base, np.int32)
        posA = self.offA[tiles[selA]] * P + slot[selA] * P + parts[selA]
        flatA[core_s[selA], posA] = col_s[selA]
        posB = self.offB[tiles[~selA]] * P + slot[~selA] * P + parts[~selA]
        flatB[core_s[~selA], posB] = col_s[~selA] - b_base
        assert flatA.max() < 32768 and flatB.max() < 32768
        self.idxA = np.stack([_wrap16(flatA[c].astype(np.int16), za)
                              for c in range(NCORES)])
        self.idxB = np.stack(
            [_wrap16(flatB[c].astype(np.int16), np.int16(zb - b_base))
             for c in range(NCORES)])


def _dma_gather_flex(gp, out_ap, in_ap, idxs_ap, num_idxs, elem_size,
                     elem_step, single_packet=False):
    """InstDMAGatherAnt with elem_size_bytes not a multiple of 256 (the ucode
    only needs the row STRIDE 256B-aligned). in_ap must be col-sliced so its
    innermost dim count == elem_size and ap[0][0] == elem_step."""
    from concourse import ap_utils
    assert idxs_ap.dtype == mybir.dt.int16
    assert in_ap.dtype == out_ap.dtype
    assert ap_utils.ap_is_contiguous(out_ap.ap[1:])
    assert ap_utils.ap_is_contiguous(idxs_ap.ap[1:])
    assert in_ap.ap[-1][1] == elem_size and in_ap.ap[0][0] == elem_step
    stride_bytes = elem_step * mybir.dt.size(in_ap.dtype)
    assert stride_bytes % 256 == 0 and stride_bytes // 256 < 256
    _in_ap = gp.lower_ap_dma(in_ap, for_custom_bir_dma=True)
    _idxs_ap = gp.lower_ap(idxs_ap)
    _out_ap = gp.lower_ap(out_ap)
    return gp.add_instruction(
        mybir.InstDMAGatherAnt(
            name=gp.bass.get_next_instruction_name(),
            ins=[*_in_ap, _idxs_ap,
                 gp.lower_val_access(gp.to_reg(num_idxs))],
            outs=[_out_ap],
            transpose=False, num_idxs=num_idxs, elem_size=elem_size,
            stride_bytes_256=stride_bytes // 256, gen_mode=0,
            single_packet=single_packet, queue_num=0,
            sbuf_tokens_per_rank=0, sbuf_free_dim_per_rank=0,
            sbuf_free_dim_pad_per_rank=0, sbuf_byte_offset=0))


def host_prep(inputs):
    fi = np.asarray(inputs["features_index"])
    N = fi.shape[0]
    VOCAB = inputs["word_emb"].shape[0]
    NFEAT = inputs["word_emb"].shape[1]
    HID = inputs["tw_W1"].shape[1]
    JOINT = inputs["tw_W2"].shape[1]
    B = inputs["tw_graph_idx"].shape[0]
    assert N == VOCAB == inputs["user_emb"].shape[0]
    assert N % NCORES == 0
    npc = N // NCORES                      # real nodes per core
    S = ((npc + P - 1) // P) * P
    assert npc < S, "need pad rows per stripe"
    ntiles = S // P
    b_base = ACORES * S

    p = dict(N=N, S=S, ntiles=ntiles, B=B, NFEAT=NFEAT, HID=HID, JOINT=JOINT,
             b_base=b_base, npc=npc)

    def number_nodes(row, col, tertiary=None):
        deg = np.bincount(row, minlength=N)
        order = np.argsort(-deg, kind="stable")
        core_of = np.empty(N, np.int64)
        core_of[order] = _snake_deal(N)
        half_a = core_of[col] < ACORES
        degA = np.bincount(row[half_a], minlength=N)
        degB = deg - degA
        ter = tertiary if tertiary is not None else np.zeros(N, np.int64)
        local = np.empty(N, np.int64)
        for c in range(NCORES):
            mine = np.flatnonzero(core_of == c)
            o = mine[np.lexsort((ter[mine], degB[mine], degA[mine]))[::-1]]
            local[o] = np.arange(o.shape[0])
        return core_of, local, core_of * S + local

    tw_row = np.asarray(inputs["tw_edges"][0])
    tw_col = np.asarray(inputs["tw_edges"][1])
    ut_row = np.asarray(inputs["ut_edges"][0])
    ut_col = np.asarray(inputs["ut_edges"][1])
    # tertiary key for tweets: word-half-A count, to tighten the word-mean
    # A/B slot rectangles within (degA, degB) groups
    wA_cnt = (fi % NCORES < ACORES).sum(axis=1).astype(np.int64)
    twc, twl, twg = number_nodes(tw_row, tw_col, tertiary=wA_cnt)
    utc, utl, utg = number_nodes(ut_row, ut_col)
    p["twc"], p["twl"], p["utc"], p["utl"] = twc, twl, utc, utl

    za, zb = 0 * S + npc, ACORES * S + npc
    p["tw_slots"] = SlotStruct(twc[tw_row], twl[tw_row], twg[tw_col],
                               S, ntiles, za, zb, b_base)
    p["ut_slots"] = SlotStruct(utc[ut_row], utl[ut_row], utg[ut_col],
                               S, ntiles, za, zb, b_base)

    w = np.arange(VOCAB)
    wcore, wlocal = w % NCORES, w // NCORES
    gw = wcore * S + wlocal
    L = fi.shape[1]
    t_rep = np.repeat(np.arange(N), L)
    p["wm_slots"] = SlotStruct(twc[t_rep], twl[t_rep], gw[fi.reshape(-1)],
                               S, ntiles, za, zb, b_base)

    word_emb = np.asarray(inputs["word_emb"], np.float32)
    user_emb = np.asarray(inputs["user_emb"], np.float32)
    # host-transposed fp16 stripes [NFEAT, S] (row-permuted per core)
    wsT = np.zeros((NCORES, NFEAT, S), np.float16)
    usT = np.zeros((NCORES, NFEAT, S), np.float16)
    for c in range(NCORES):
        sel = np.flatnonzero(wcore == c)
        wsT[c][:, wlocal[sel]] = word_emb[sel].T.astype(np.float16)
        sel = np.flatnonzero(utc == c)
        usT[c][:, utl[sel]] = user_emb[sel].T.astype(np.float16)

    def fold1(W1, a1):
        h = W1.shape[1]
        return np.concatenate(
            [W1, W1 @ a1[h:, None], W1 @ a1[:h, None]], axis=1)

    p["tw_W1f"] = fold1(np.asarray(inputs["tw_W1"]),
                        np.asarray(inputs["tw_a1"])).astype(np.float16)
    p["tu_W1f"] = fold1(np.asarray(inputs["tu_W1"]),
                        np.asarray(inputs["tu_a1"])).astype(np.float16)
    p["tw_W2f"] = fold1(np.asarray(inputs["tw_W2"]),
                        np.asarray(inputs["tw_a2"])).astype(np.float16)
    p["tu_W2f"] = fold1(np.asarray(inputs["tu_W2"]),
                        np.asarray(inputs["tu_a2"])).astype(np.float16)
    p["weight_W"] = np.asarray(inputs["weight_W"]).astype(np.float16)
    p["projT"] = np.asarray(inputs["weight_proj"]).reshape(1, JOINT).astype(np.float32)
    p["out_WT"] = np.asarray(inputs["out_W"]).T.astype(np.float16)

    twi = np.asarray(inputs["tw_graph_idx"])
    uti = np.asarray(inputs["ut_graph_idx"])
    BT = ((B + P - 1) // P) * P + P
    p["BT"] = BT
    bt_tiles = BT // P
    p["bt_tiles"] = bt_tiles
    # bias tile replicated for the batched log_softmax pass
    p["obt"] = np.tile(np.asarray(inputs["out_b"], np.float32).reshape(1, 2),
                       (P, bt_tiles)).astype(np.float32)
    u_max = 1
    owns = []
    for c in range(NCORES):
        own = np.flatnonzero((twc[twi] == c) | (utc[uti] == c))
        owns.append(own)
        u_max = max(u_max, (own.shape[0] + P - 1) // P)
    p["u_fus"] = u_max
    g_tw = np.zeros((NCORES, 16, u_max * 8), np.int16)
    g_tu = np.zeros((NCORES, 16, u_max * 8), np.int16)
    sc_idx = np.zeros((NCORES, 128, u_max), np.int32)
    for c in range(NCORES):
        own = owns[c]
        n = own.shape[0]
        ftw = np.full(u_max * P, npc, np.int32)
        ftu = np.full(u_max * P, npc, np.int32)
        pos = np.arange(n)
        sel = twc[twi[own]] == c
        ftw[pos[sel]] = twl[twi[own[sel]]]
        sel = utc[uti[own]] == c
        ftu[pos[sel]] = utl[uti[own[sel]]]
        g_tw[c] = _wrap16(ftw.astype(np.int16), np.int16(npc))
        g_tu[c] = _wrap16(ftu.astype(np.int16), np.int16(npc))
        sc = B + np.tile(np.arange(P), u_max)
        sc[pos] = own
        sc_idx[c] = sc.reshape(u_max, P).T
    p["fus_gtw"], p["fus_gtu"], p["fus_sc"] = g_tw, g_tu, sc_idx

    # ---- pack all inputs into two per-core blobs (one 2-byte, one 4-byte):
    # the axon-tunneled PJRT path costs ~1.7ms per input tensor per call,
    # so input COUNT dominates the measured time. Pieces live as [rows, cols]
    # blocks at column offsets inside blob16 [128, W16] / blob32 [128, W32].
    kchunks = [(i, min(P, NFEAT - i)) for i in range(0, NFEAT, P)]
    L16, off = {}, 0

    def add16(key, rows, cols):
        nonlocal off
        L16[key] = (rows, cols, off)
        off += cols
    for ki, (k0, kn) in enumerate(kchunks):
        add16(f"wsT{ki}", kn, S)
    for ki, (k0, kn) in enumerate(kchunks):
        add16(f"usT{ki}", kn, S)
    for ki, (k0, kn) in enumerate(kchunks):
        add16(f"twW1f{ki}", kn, HID + 2)
    for ki, (k0, kn) in enumerate(kchunks):
        add16(f"tuW1f{ki}", kn, HID + 2)
    add16("twW2f", HID, JOINT + 2)
    add16("tuW2f", HID, JOINT + 2)
    add16("weightW", JOINT, JOINT)
    add16("outWT", JOINT, 2)
    # 16-row pieces stack vertically, 8 bands of 16 rows in shared columns
    idx_keys = [("wmA", p["wm_slots"].idxA.shape[2]),
                ("wmB", p["wm_slots"].idxB.shape[2]),
                ("twA", p["tw_slots"].idxA.shape[2]),
                ("twB", p["tw_slots"].idxB.shape[2]),
                ("utA", p["ut_slots"].idxA.shape[2]),
                ("utB", p["ut_slots"].idxB.shape[2]),
                ("fgtw", u_max * 8), ("fgtu", u_max * 8)]
    band_w = max(w for _, w in idx_keys)
    for bi, (key, w) in enumerate(idx_keys):
        L16[key] = (16, w, off, bi * 16)   # (rows, cols, col_off, row_off)
    off += band_w
    W16 = off
    L32, off = {}, 0

    def add32(key, rows, cols):
        nonlocal off
        L32[key] = (rows, cols, off)
        off += cols
    add32("projT", 1, JOINT)
    add32("obt", P, bt_tiles * 2)
    add32("fussc", 128, u_max)
    W32 = off
    p["L16"], p["W16"], p["L32"], p["W32"] = L16, W16, L32, W32

    blob16 = np.zeros((NCORES, 128, W16), np.int16)
    blob32 = np.zeros((NCORES, 128, W32), np.int32)

    def put16(c, key, arr):
        ent = L16[key]
        r, w, o = ent[0], ent[1], ent[2]
        r0 = ent[3] if len(ent) > 3 else 0
        assert arr.shape == (r, w), (key, arr.shape, (r, w))
        blob16[c, r0:r0 + r, o:o + w] = arr.view(np.int16)

    def put32(c, key, arr):
        r, w, o = L32[key]
        assert arr.shape == (r, w), (key, arr.shape, (r, w))
        blob32[c, :r, o:o + w] = arr.view(np.int32)

    for c in range(NCORES):
        for ki, (k0, kn) in enumerate(kchunks):
            put16(c, f"wsT{ki}", wsT[c][k0:k0 + kn])
            put16(c, f"usT{ki}", usT[c][k0:k0 + kn])
            put16(c, f"twW1f{ki}", p["tw_W1f"][k0:k0 + kn])
            put16(c, f"tuW1f{ki}", p["tu_W1f"][k0:k0 + kn])
        put16(c, "twW2f", p["tw_W2f"])
        put16(c, "tuW2f", p["tu_W2f"])
        put16(c, "weightW", p["weight_W"])
        put16(c, "outWT", p["out_WT"])
        put16(c, "wmA", p["wm_slots"].idxA[c])
        put16(c, "wmB", p["wm_slots"].idxB[c])
        put16(c, "twA", p["tw_slots"].idxA[c])
        put16(c, "twB", p["tw_slots"].idxB[c])
        put16(c, "utA", p["ut_slots"].idxA[c])
        put16(c, "utB", p["ut_slots"].idxB[c])
        put16(c, "fgtw", p["fus_gtw"][c])
        put16(c, "fgtu", p["fus_gtu"][c])
        put32(c, "projT", p["projT"])
        put32(c, "obt", p["obt"])
        put32(c, "fussc", p["fus_sc"][c])
    p["blob16"], p["blob32"] = blob16, blob32
    return p


def build_program(p):
    import concourse.bacc as bacc
    nc_b = bacc.Bacc("TRN2", target_bir_lowering=False, debug=False,
                     num_devices=NCORES)
    tcx = tile.TileContext(nc_b)
    S, ntiles, B, BT = p["S"], p["ntiles"], p["B"], p["BT"]
    NFEAT, HID, JOINT, N = p["NFEAT"], p["HID"], p["JOINT"], p["N"]
    b_base, npc = p["b_base"], p["npc"]
    NT = NCORES * S
    DW, DL2 = HID * 2, JOINT * 2
    DC1, DC2 = 2 * DW, 2 * DL2           # merged-table row widths
    u_fus = p["u_fus"]
    bt_tiles = p["bt_tiles"]
    BT2 = BT + P
    wm, tws, uts = p["wm_slots"], p["tw_slots"], p["ut_slots"]
    npad = S - npc
    kchunks = [(i, min(P, NFEAT - i)) for i in range(0, NFEAT, P)]
    nk = len(kchunks)
    # embeddings streamed in two tile-aligned half-stripes to cap SBUF use
    th_splits = [(0, (ntiles + 1) // 2), ((ntiles + 1) // 2, ntiles)]
    Shmax = max(t1 - t0 for t0, t1 in th_splits) * P

    with tcx as tc:
        nc = tc.nc
        ctx = ExitStack()

        def inp(name, shape, dtype):
            return nc.dram_tensor(name, shape, dtype, kind="ExternalInput").ap()

        def internal(name, shape, dtype, shared=False):
            return nc.dram_tensor(
                name, shape, dtype, kind="Internal",
                addr_space="Shared" if shared else "Local").ap()

        blob16 = inp("blob16", [128, p["W16"]], I16)
        blob32 = inp("blob32", [128, p["W32"]], I32)
        L16, L32 = p["L16"], p["L32"]

        def b16(key, dt=F16):
            ent = L16[key]
            r, w, o = ent[0], ent[1], ent[2]
            r0 = ent[3] if len(ent) > 3 else 0
            ap = blob16[r0:r0 + r, o:o + w]
            return ap if dt == I16 else ap.bitcast(dt)

        def b32(key, dt=F32):
            r, w, o = L32[key]
            ap = blob32[0:r, o:o + w]
            return ap if dt == I32 else ap.bitcast(dt)

        out = nc.dram_tensor("out", [B, 2], F32, kind="ExternalOutput").ap()

        # merged tables: comb1 rows = [word row (128) | user-L1 row (128)],
        # comb2 rows = [ut-L2 row (256) | tw-L2 row (256)]
        comb1_stripe = internal("comb1_s", [S, DC1], F16)
        comb1_table = internal("comb1_t", [NT, DC1], F16, shared=True)
        t1tw_stripe = internal("t1tw_s", [S, DW], F16)
        t1tw_table = internal("t1tw_t", [NT, DW], F16, shared=True)
        comb2_stripe = internal("comb2_s", [S, DC2], F16)
        comb2_table = internal("comb2_t", [NT, DC2], F16, shared=True)
        x_stripe = {g: internal(f"{g}_x", [S, JOINT], F16) for g in ("tw", "ut")}
        lbuf = internal("lbuf", [BT2, 4], F32)
        lbuf_r = internal("lbuf_r", [BT2, 4], F32, shared=True)

        rg = [list(range(NCORES))]

        big = ctx.enter_context(tc.tile_pool(name="big", bufs=2))
        med = ctx.enter_context(tc.tile_pool(name="med", bufs=4))
        sml = ctx.enter_context(tc.tile_pool(name="sml", bufs=6))
        wide = ctx.enter_context(tc.tile_pool(name="wide", bufs=1))
        fsp = ctx.enter_context(tc.tile_pool(name="fsp", bufs=8))
        pst = ctx.enter_context(tc.tile_pool(name="pst", bufs=2, space="PSUM"))
        psm = ctx.enter_context(tc.tile_pool(name="psm", bufs=2, space="PSUM"))
        acc = ctx.enter_context(tc.tile_pool(name="acc", bufs=1, space="PSUM"))
        cst = ctx.enter_context(tc.tile_pool(name="cst", bufs=1))
        idxall = ctx.enter_context(tc.tile_pool(name="idxall", bufs=1))

        ident16 = cst.tile([P, P], F16, tag="ident16")
        make_identity(nc, ident16[:])
        ones_col = cst.tile([P, 1], F16, tag="ones_col")
        nc.vector.memset(ones_col[:], 1.0)
        ones_row = cst.tile([1, P], F16, tag="ones_row")
        nc.vector.memset(ones_row[:], 1.0)
        padfd = cst.tile([P, 1], F16, tag="padfd")
        nc.vector.memset(padfd[:], PAD_FD)

        def expand128(dst, src_ap, w):
            # replicate [16, w] int16 into [128, w] via doubling DMAs
            nc.sync.dma_start(dst[0:16, 0:w], src_ap)
            nc.sync.dma_start(dst[16:32, 0:w], dst[0:16, 0:w])
            nc.sync.dma_start(dst[32:64, 0:w], dst[0:32, 0:w])
            nc.sync.dma_start(dst[64:128, 0:w], dst[0:64, 0:w])

        def load_idx(slots, keyA, keyB, tagsfx=""):
            wa = int(slots.offA[-1]) * 8
            wb = int(slots.offB[-1]) * 8
            ia = idxall.tile([P, max(wa, 8)], I16, tag="ia_all",
                             name="ia_all" + tagsfx)
            if wa > 0:
                expand128(ia, b16(keyA, I16)[:, 0:wa], wa)
            ib = idxall.tile([P, max(wb, 8)], I16, tag="ib_all",
                             name="ib_all" + tagsfx)
            if wb > 0:
                expand128(ib, b16(keyB, I16)[:, 0:wb], wb)
            return ia, ib

        def slot_gather(slots, ia, ib, table, col0, dtab, dg, t):
            # dg = payload elems per row; rows start at column col0 of the
            # dtab-elem-wide table rows
            KA, KB = int(slots.KA[t]), int(slots.KB[t])
            K = max(KA + KB, 1)
            g_t = big.tile([P, K, dg], F16, tag="g_e", bufs=2)
            if KA + KB == 0:
                nc.vector.memset(g_t[:], 0.0)
                return g_t, K
            if KA > 0:
                _dma_gather_flex(
                    nc.gpsimd, g_t[:, 0:KA, :],
                    table[0:b_base, col0:col0 + dg],
                    ia[:, int(slots.offA[t]) * 8:int(slots.offA[t] + KA) * 8],
                    KA * P, dg, dtab, single_packet=(KA * P <= 1024))
            if KB > 0:
                _dma_gather_flex(
                    nc.gpsimd, g_t[:, KA:KA + KB, :],
                    table[b_base:, col0:col0 + dg],
                    ib[:, int(slots.offB[t]) * 8:int(slots.offB[t] + KB) * 8],
                    KB * P, dg, dtab, single_packet=(KB * P <= 1024))
            return g_t, K

        def elu_batched(buf_ap, nelem):
            # in-place ELU on an f16 [P, nelem] view
            m16 = med.tile([P, nelem], F16, tag="elu_m", bufs=1)
            nc.vector.tensor_scalar_min(m16[:], buf_ap, 0.0)
            nc.scalar.activation(m16[:], m16[:],
                                 mybir.ActivationFunctionType.Exp)
            nc.vector.tensor_scalar_add(m16[:], m16[:], -1.0)
            nc.vector.tensor_tensor(buf_ap, buf_ap, m16[:],
                                    op=mybir.AluOpType.max)

        def stripe_write(stripe, rows_ap, d):
            # one DMA: SBUF rows [P, ntiles, d] -> DRAM stripe [S, d]
            nc.sync.dma_start(
                stripe.rearrange("(t q) d -> q t d", q=P),
                rows_ap.rearrange("q (t d) -> q t d", d=d))

        # ===== phase 1: word + user-L1 rows -> merged comb1 table =====
        c1rows = wide.tile([P, ntiles * DC1], F16, tag="cX", name="c1rows")
        nc.vector.memset(c1rows[:], 0.0)
        fs1 = {}
        embp_ctx = ExitStack()
        embp = embp_ctx.enter_context(tc.tile_pool(name="embp", bufs=1))
        for g, ekey, wkey, col0, store_fs in (
                ("w", "wsT", "twW1f", 0, False),
                ("ut", "usT", "tuW1f", DW, True)):
            ncols = HID + 2
            wt = cst.tile([P, ncols * nk], F16, tag=f"wf_{wkey}")
            for ki, (k0, kn) in enumerate(kchunks):
                nc.sync.dma_start(wt[:kn, ki * ncols:(ki + 1) * ncols],
                                  b16(f"{wkey}{ki}"))
            if store_fs:
                fs1["ut"] = fsp.tile([P, ntiles], F32, tag="fs",
                                     name="fs1_ut")
            ncopy = ncols if not store_fs else ncols - 1
            for h, (t0, t1) in enumerate(th_splits):
                Swin = (t1 - t0) * P
                embT_sb = embp.tile([P, nk * Shmax], F16, tag="embT",
                                    name=f"embT_{g}{h}")
                for ki, (k0, kn) in enumerate(kchunks):
                    nc.sync.dma_start(
                        embT_sb[:kn, ki * Shmax:ki * Shmax + Swin],
                        b16(f"{ekey}{ki}")[:, t0 * P:t1 * P])
                for t in range(t0, t1):
                    tl = (t - t0) * P
                    ps = psm.tile([P, ncols], F32, tag="mm")
                    for ki, (k0, kn) in enumerate(kchunks):
                        nc.tensor.matmul(
                            ps[:],
                            embT_sb[:kn, ki * Shmax + tl:ki * Shmax + tl + P],
                            wt[:kn, ki * ncols:(ki + 1) * ncols],
                            start=(ki == 0), stop=(ki == nk - 1))
                    nc.vector.tensor_copy(
                        c1rows[:, t * DC1 + col0:t * DC1 + col0 + ncopy],
                        ps[:, 0:ncopy])
                    if store_fs:
                        nc.vector.tensor_copy(fs1["ut"][:, t:t + 1],
                                              ps[:, ncols - 1:ncols])
        embp_ctx.close()
        stripe_write(comb1_stripe, c1rows[:], DC1)
        nc.sync.dma_start(comb1_stripe[npc:S, DW + HID:DW + HID + 1],
                          padfd[:npad, :])
        nc.gpsimd.collective_compute("AllGather", mybir.AluOpType.bypass, rg,
                                     ins=[comb1_stripe[:]],
                                     outs=[comb1_table[:]])

        # ===== phase 2: tweet means -> tweet L1 stripe (own AllGather) =====
        wm_ia, wm_ib = load_idx(wm, "wmA", "wmB", "w")
        fs1["tw"] = fsp.tile([P, ntiles], F32, tag="fs", name="fs1_tw")
        rows_t = wide.tile([P, ntiles * DW], F16, tag="w12", name="rows_tw1")
        nc.vector.memset(rows_t[:], 0.0)
        for t in range(ntiles):
            g_t, K = slot_gather(wm, wm_ia, wm_ib, comb1_table, 0, DC1,
                                 HID + 2, t)
            mean = med.tile([P, HID + 2], F32, tag="wm_mean")
            nc.vector.tensor_reduce(
                mean[:], g_t[:].rearrange("q k d -> q d k"),
                axis=mybir.AxisListType.X, op=mybir.AluOpType.add)
            nc.vector.tensor_scalar_mul(
                rows_t[:, t * DW:t * DW + HID + 1], mean[:, :HID + 1],
                1.0 / 16.0)
            nc.vector.tensor_scalar_mul(fs1["tw"][:, t:t + 1],
                                        mean[:, HID + 1:HID + 2], 1.0 / 16.0)
        stripe_write(t1tw_stripe, rows_t[:], DW)
        nc.sync.dma_start(t1tw_stripe[npc:S, HID:HID + 1], padfd[:npad, :])
        nc.gpsimd.collective_compute("AllGather", mybir.AluOpType.bypass, rg,
                                     ins=[t1tw_stripe[:]],
                                     outs=[t1tw_table[:]])

        # ===== phase 3: both graphs' L1 pass -> merged comb2 table =====
        w2tiles = {}
        for g, w2key in (("ut", "tuW2f"), ("tw", "twW2f")):
            wt2 = cst.tile([P, JOINT + 2], F16, tag=f"w2_{g}")
            nc.sync.dma_start(wt2[:HID, :], b16(w2key))
            w2tiles[g] = wt2
        wwt = cst.tile([P, JOINT], F16, tag="wwt")
        nc.sync.dma_start(wwt[:], b16("weightW"))
        projs = cst.tile([1, JOINT], F32, tag="projs")
        nc.sync.dma_start(projs[:], b32("projT"))
        colsum = {g: acc.tile([1, JOINT], F32, tag=f"cs_{g}", name=f"cs_{g}")
                  for g in ("ut", "tw")}

        def attn_pass(slots, ia, ib, table, col0, dtab, d_in, fs_t, den_t,
                      num_t):
            for t in range(ntiles):
                g_t, K = slot_gather(slots, ia, ib, table, col0, dtab,
                                     d_in + 2, t)
                lr = med.tile([P, K], F32, tag="lr_e")
                fd_view = g_t[:, :, d_in:d_in + 1].rearrange(
                    "q k o -> q (k o)")
                nc.scalar.activation(lr[:], fd_view,
                                     mybir.ActivationFunctionType.Lrelu,
                                     bias=fs_t[:, t:t + 1], scale=1.0,
                                     alpha=ALPHA)
                e_t = med.tile([P, K], F16, tag="e_e")
                nc.scalar.activation(e_t[:], lr[:],
                                     mybir.ActivationFunctionType.Exp,
                                     scale=-1.0,
                                     accum_out=den_t[:, t:t + 1])
                nc.vector.tensor_tensor(g_t[:, :, 0:d_in],
                                        g_t[:, :, 0:d_in],
                                        e_t[:].to_broadcast([P, K, d_in]),
                                        op=mybir.AluOpType.mult)
                nc.vector.tensor_reduce(
                    num_t[:, t * d_in:(t + 1) * d_in],
                    g_t[:, :, 0:d_in].rearrange("q k d -> q d k"),
                    axis=mybir.AxisListType.X, op=mybir.AluOpType.add)

        def finish_o16(den_t, num_t, d_in, o16_t):
            nc.vector.tensor_scalar_add(den_t[:], den_t[:], EPS)
            nc.vector.reciprocal(den_t[:], den_t[:])
            for t in range(ntiles):
                nc.vector.tensor_scalar_mul(
                    o16_t[:, t * d_in:(t + 1) * d_in],
                    num_t[:, t * d_in:(t + 1) * d_in], den_t[:, t:t + 1])
            elu_batched(o16_t[:], ntiles * d_in)

        c2rows = wide.tile([P, ntiles * DC2], F16, tag="cX", name="c2rows")
        nc.vector.memset(c2rows[:], 0.0)
        eidx = {}
        fs2 = {}
        for g, keyA, keyB, slots, t1c0, t1tab, t1d in (
                ("ut", "utA", "utB", uts, DW, comb1_table, DC1),
                ("tw", "twA", "twB", tws, 0, t1tw_table, DW)):
            c2off = 0 if g == "ut" else DL2
            e_ia, e_ib = load_idx(slots, keyA, keyB, "e" + g)
            eidx[g] = (keyA, keyB)
            den1 = fsp.tile([P, ntiles], F32, tag="fs", name=f"den1_{g}")
            num1 = wide.tile([P, ntiles * HID], F32, tag="w12",
                             name=f"num1_{g}")
            attn_pass(slots, e_ia, e_ib, t1tab, t1c0, t1d, HID,
                      fs1[g], den1, num1)
            o16 = wide.tile([P, ntiles * HID], F16, tag="oX",
                            name=f"o16_{g}")
            finish_o16(den1, num1, HID, o16)
            fs2[g] = fsp.tile([P, ntiles], F32, tag="fs", name=f"fs2_{g}")
            for t in range(ntiles):
                tp = pst.tile([P, P], F16, tag="tp")
                nc.tensor.transpose(tp[:HID, :],
                                    o16[:, t * HID:(t + 1) * HID],
                                    ident16[:])
                tp16 = med.tile([P, P], F16, tag="tp16")
                nc.vector.tensor_copy(tp16[:HID, :], tp[:HID, :])
                ps2 = psm.tile([P, JOINT + 2], F32, tag="mm")
                nc.tensor.matmul(ps2[:], tp16[:HID, :], w2tiles[g][:HID, :],
                                 start=True, stop=True)
                nc.vector.tensor_copy(
                    c2rows[:, t * DC2 + c2off:t * DC2 + c2off + JOINT + 1],
                    ps2[:, :JOINT + 1])
                nc.vector.tensor_copy(fs2[g][:, t:t + 1],
                                      ps2[:, JOINT + 1:JOINT + 2])
        stripe_write(comb2_stripe, c2rows[:], DC2)
        nc.sync.dma_start(comb2_stripe[npc:S, JOINT:JOINT + 1],
                          padfd[:npad, :])
        nc.sync.dma_start(comb2_stripe[npc:S, DL2 + JOINT:DL2 + JOINT + 1],
                          padfd[:npad, :])
        nc.gpsimd.collective_compute("AllGather", mybir.AluOpType.bypass, rg,
                                     ins=[comb2_stripe[:]],
                                     outs=[comb2_table[:]])

        # ===== phase 4: both graphs' L2 pass =====
        for g, slots in (("ut", uts), ("tw", tws)):
            c2off = 0 if g == "ut" else DL2
            keyA, keyB = eidx[g]
            e_ia, e_ib = load_idx(slots, keyA, keyB, "e2" + g)
            den2 = fsp.tile([P, ntiles], F32, tag="fs", name=f"den2_{g}")
            num2 = wide.tile([P, ntiles * JOINT], F32, tag="w25",
                             name=f"num2_{g}")
            attn_pass(slots, e_ia, e_ib, comb2_table, c2off, DC2, JOINT,
                      fs2[g], den2, num2)
            x16 = wide.tile([P, ntiles * JOINT], F16, tag="oX",
                            name=f"x16_{g}")
            finish_o16(den2, num2, JOINT, x16)
            stripe_write(x_stripe[g], x16[:], JOINT)
            for t in range(ntiles):
                tp = pst.tile([P, P], F16, tag="tp")
                nc.tensor.transpose(tp[:], x16[:, t * JOINT:(t + 1) * JOINT],
                                    ident16[:])
                tp16 = med.tile([P, P], F16, tag="tp16")
                nc.vector.tensor_copy(tp16[:], tp[:])
                ups = psm.tile([P, JOINT], F32, tag="mm")
                nc.tensor.matmul(ups[:], tp16[:], wwt[:], start=True,
                                 stop=True)
                th = med.tile([P, JOINT], F16, tag="tanh")
                nc.scalar.activation(th[:], ups[:],
                                     mybir.ActivationFunctionType.Tanh)
                nc.tensor.matmul(colsum[g][:], ones_col[:], th[:],
                                 start=(t == 0), stop=(t == ntiles - 1),
                                 skip_group_check=True)

        # ===== phase 5: attention numerators (local partials) =====
        attp = sml.tile([1, 2], F32, tag="attp")
        for gi, g in enumerate(("tw", "ut")):
            prod = sml.tile([1, JOINT], F32, tag="pr", bufs=1)
            nc.vector.tensor_tensor(prod[:], colsum[g][:], projs[:],
                                    op=mybir.AluOpType.mult)
            nc.vector.tensor_reduce(attp[:, gi:gi + 1], prod[:],
                                    axis=mybir.AxisListType.X,
                                    op=mybir.AluOpType.add)
        nc.vector.tensor_scalar_mul(attp[:], attp[:], 1.0 / N)

        # ===== phase 6: per-view partial logits + att row -> ONE AllReduce
        zt = med.tile([P, (bt_tiles + 1) * 4], F32, tag="zt", bufs=1)
        nc.vector.memset(zt[:], 0.0)
        nc.sync.dma_start(lbuf.rearrange("(t q) d -> q t d", q=P),
                          zt[:].rearrange("q (t d) -> q t d", d=4))
        fgw = sml.tile([P, u_fus * 8], I16, tag="fgw")
        expand128(fgw, b16("fgtw", I16), u_fus * 8)
        fgu = sml.tile([P, u_fus * 8], I16, tag="fgu")
        expand128(fgu, b16("fgtu", I16), u_fus * 8)
        g1 = big.tile([P, u_fus, JOINT], F16, tag="fg1", bufs=1)
        nc.gpsimd.dma_gather(g1[:], x_stripe["tw"][:], fgw[:], u_fus * P,
                             u_fus * P, JOINT,
                             single_packet=(u_fus * P <= 1024))
        g2 = big.tile([P, u_fus, JOINT], F16, tag="fg2", bufs=1)
        nc.gpsimd.dma_gather(g2[:], x_stripe["ut"][:], fgu[:], u_fus * P,
                             u_fus * P, JOINT,
                             single_packet=(u_fus * P <= 1024))
        owt = cst.tile([P, 2], F16, tag="owt")
        nc.sync.dma_start(owt[:JOINT, :], b16("outWT"))
        sct = sml.tile([P, u_fus], I32, tag="fsct")
        nc.sync.dma_start(sct[:], b32("fussc", I32))
        for j in range(u_fus):
            lgs4 = sml.tile([P, 4], F32, tag="lgs4")
            for col, gsrc in ((0, g1), (2, g2)):
                tp = pst.tile([P, P], F16, tag="tp")
                nc.tensor.transpose(tp[:], gsrc[:, j, :], ident16[:])
                tp16 = med.tile([P, P], F16, tag="tp16")
                nc.vector.tensor_copy(tp16[:], tp[:])
                lg = psm.tile([P, 2], F32, tag="mm2")
                nc.tensor.matmul(lg[:], tp16[:JOINT, :], owt[:JOINT, :],
                                 start=True, stop=True)
                nc.vector.tensor_copy(lgs4[:, col:col + 2], lg[:])
            nc.gpsimd.indirect_dma_start(
                out=lbuf[:],
                out_offset=bass.IndirectOffsetOnAxis(ap=sct[:, j:j + 1],
                                                     axis=0),
                in_=lgs4[:], in_offset=None)
        nc.sync.dma_start(lbuf[BT:BT + 1, 0:2], attp[:])
        nc.gpsimd.collective_compute("AllReduce", mybir.AluOpType.add, rg,
                                     ins=[lbuf[:]], outs=[lbuf_r[:]])

        # ===== phase 7: att softmax + mix + bias + log_softmax, 1 out DMA
        atts = sml.tile([1, 2], F32, tag="atts")
        nc.sync.dma_start(atts[:], lbuf_r[BT:BT + 1, 0:2])
        mx = sml.tile([1, 1], F32, tag="attmx")
        nc.vector.tensor_reduce(mx[:], atts[:], axis=mybir.AxisListType.X,
                                op=mybir.AluOpType.max)
        sh = sml.tile([1, 2], F32, tag="attsh")
        nc.vector.tensor_scalar(sh[:], atts[:], mx[:], None,
                                op0=mybir.AluOpType.subtract)
        ex = sml.tile([1, 2], F32, tag="attex")
        nc.scalar.activation(ex[:], sh[:], mybir.ActivationFunctionType.Exp)
        sm = sml.tile([1, 1], F32, tag="attsm")
        nc.vector.tensor_reduce(sm[:], ex[:], axis=mybir.AxisListType.X,
                                op=mybir.AluOpType.add)
        nc.vector.reciprocal(sm[:], sm[:])
        att2 = sml.tile([1, 2], F16, tag="att2")
        nc.vector.tensor_scalar_mul(att2[:], ex[:], sm[:])
        attb_ps = psm.tile([P, 2], F32, tag="mm2")
        nc.tensor.matmul(attb_ps[:], ones_row[:], att2[:], start=True,
                         stop=True)
        attb = sml.tile([P, 2], F32, tag="attb")
        nc.vector.tensor_copy(attb[:], attb_ps[:])

        lgall = med.tile([P, bt_tiles, 4], F32, tag="lgall", bufs=1)
        nc.sync.dma_start(
            lgall[:],
            lbuf_r.rearrange("(t q) d -> q t d", q=P)[:, 0:bt_tiles, :])
        fin = med.tile([P, bt_tiles, 2], F32, tag="fin")
        nc.vector.tensor_scalar_mul(fin[:], lgall[:, :, 0:2], attb[:, 0:1])
        fin2 = med.tile([P, bt_tiles, 2], F32, tag="fin2")
        nc.vector.tensor_scalar_mul(fin2[:], lgall[:, :, 2:4], attb[:, 1:2])
        nc.vector.tensor_tensor(fin[:], fin[:], fin2[:],
                                op=mybir.AluOpType.add)
        obt = sml.tile([P, bt_tiles * 2], F32, tag="obt", bufs=1)
        nc.sync.dma_start(obt[:], b32("obt"))
        nc.vector.tensor_tensor(
            fin[:], fin[:], obt[:].rearrange("q (t d) -> q t d", d=2),
            op=mybir.AluOpType.add)
        m = med.tile([P, bt_tiles], F32, tag="lgm")
        nc.vector.tensor_reduce(m[:], fin[:], axis=mybir.AxisListType.X,
                                op=mybir.AluOpType.max)
        shl = med.tile([P, bt_tiles, 2], F32, tag="lgsh")
        nc.vector.tensor_tensor(shl[:], fin[:],
                                m[:].to_broadcast([P, bt_tiles, 2]),
                                op=mybir.AluOpType.subtract)
        exl = med.tile([P, bt_tiles, 2], F32, tag="lgex")
        nc.scalar.activation(exl[:], shl[:],
                             mybir.ActivationFunctionType.Exp)
        se = med.tile([P, bt_tiles], F32, tag="lgse")
        nc.vector.tensor_reduce(se[:], exl[:], axis=mybir.AxisListType.X,
                                op=mybir.AluOpType.add)
        ln = med.tile([P, bt_tiles], F32, tag="lgln")
        nc.scalar.activation(ln[:], se[:], mybir.ActivationFunctionType.Ln)
        res = med.tile([P, bt_tiles, 2], F32, tag="lgres")
        nc.vector.tensor_tensor(res[:], shl[:],
                                ln[:].to_broadcast([P, bt_tiles, 2]),
                                op=mybir.AluOpType.subtract)
        nc.sync.dma_start(out.rearrange("(t q) d -> q t d", q=P),
                          res[:, 0:B // P, :])

        ctx.close()
    return tcx


def _in_maps(p):
    return [{"blob16": p["blob16"][c], "blob32": p["blob32"][c]}
            for c in range(NCORES)]


def kernel(**inputs):
    from concourse import bass_utils
    p = host_prep(inputs)
    tcx = build_program(p)
    tcx.nc.compile()
    res = bass_utils.run_bass_kernel_spmd(tcx.nc, _in_maps(p),
                                          core_ids=list(range(NCORES)))
    return np.asarray(res.results[0]["out"], np.float32)
